# revision 13
# baseline (speedup 1.0000x reference)
"""Trainium2 Bass kernel for nn_BigramHash: out = tab[hash(t,prev)] @ w_proj.T.

Strategy (fold sharded by table rows, tokens routed to row owners):
  - The projection is folded into the table on-device:
        tab2 = tab @ w_proj.T
    sharded by table rows: core c computes rows [c*384, (c+1)*384).
  - The host routes each token to the core that owns its hashed row
    (the hash is recomputed on-device for the actual gather; the host
    copy is only the sharding function), padding each shard to a
    common capacity. Tokens are sorted by table row, which (a) gives
    the gather DRAM row-buffer locality and (b) lets early gather
    tiles depend only on the first fold chunks (range-based dep
    tracking), overlapping the gather with the fold.
  - Each core computes the bigram hash for its tokens on DVE (exact in
    fp32: all intermediates < 2^24), rebases it into its local slice,
    gathers rows with indirect DMA (128 rows x 4KB per instruction)
    and streams them to the output.
  - The host scatters per-core output rows back to token order.

Everything numeric from the reference (hash, fold matmul, gather) runs
on device; host work is sharding/layout marshalling and the routing
permutation. Output is bit-exact vs the fp32 reference.
"""

import numpy as np

import concourse.bass as bass
import concourse.tile as tile
from concourse import bacc, mybir
from concourse.bass_utils import run_bass_kernel_spmd

N_CORES = 8
B, T = 4, 8192
SZ, D = 3072, 1024
NTOK = B * T                      # 32768
SLICE = SZ // N_CORES             # 384 table rows per core
RC_LOC = SLICE // 128             # 3 fold row-chunks per core
KC = D // 128                     # 8 contraction chunks

C_T = 31337 % SZ                  # 617
C_P = 1000003 % SZ                # 1603

_CACHE = {}


def declare_io(nc, tiles):
    f32 = mybir.dt.float32
    i32 = mybir.dt.int32
    t_ap = nc.dram_tensor("t_sh", [128, tiles], i32, kind="ExternalInput").ap()
    tp_ap = nc.dram_tensor("tp_sh", [128, tiles], i32, kind="ExternalInput").ap()
    base_ap = nc.dram_tensor("base", [128, 1], f32, kind="ExternalInput").ap()
    tabT_ap = nc.dram_tensor(
        "tabT", [KC, 128, SLICE], f32, kind="ExternalInput"
    ).ap()
    wT_ap = nc.dram_tensor("w_projT", [KC, 128, D], f32, kind="ExternalInput").ap()
    out_ap = nc.dram_tensor("out_sh", [tiles * 128, D], f32, kind="ExternalOutput").ap()
    tab2_ap = nc.dram_tensor("tab2", [SLICE, D], f32).ap()
    return t_ap, tp_ap, base_ap, tabT_ap, wT_ap, out_ap, tab2_ap


def emit_body(nc, tc, io, tiles, bmax=None, do_fold=True, do_gather=True,
              gather_bufs=4, out_chunk=1):
    f32 = mybir.dt.float32
    i32 = mybir.dt.int32
    t_ap, tp_ap, base_ap, tabT_ap, wT_ap, out_ap, tab2_ap = io
    with (
        tc.tile_pool(name="weights", bufs=1) as wpool,
        tc.tile_pool(name="fold_out", bufs=3) as fpool,
        tc.tile_pool(name="psum", bufs=2, space="PSUM") as ppool,
        tc.tile_pool(name="idx", bufs=1) as ipool,
        tc.tile_pool(name="gather", bufs=gather_bufs) as gpool,
    ):
        # ---- load tabT slice (1.5 MB) and w_projT (4 MB) into SBUF ----
        tabT_sb = []
        wT_sb = []
        for kc in range(KC):
            tt = wpool.tile([128, SLICE], f32, tag=f"tabT{kc}")
            nc.scalar.dma_start(tt[:], tabT_ap[kc])
            tabT_sb.append(tt)
            wt = wpool.tile([128, D], f32, tag=f"wT{kc}")
            nc.scalar.dma_start(wt[:], wT_ap[kc])
            wT_sb.append(wt)

        # ---- hash indices on DVE ----
        # No integer mod in the TRN2 DVE ISA; compute x % SZ exactly in
        # fp32 (all intermediates < 2^24): q = int(x/SZ) may be off by
        # one in either direction, two masked corrections fix it.
        def mod_sz(dst, src):
            m = ipool.tile([128, tiles], f32, tag="mod_m")
            nc.vector.tensor_scalar(
                m[:], src[:], 1.0 / SZ, None, op0=mybir.AluOpType.mult
            )
            qi = ipool.tile([128, tiles], i32, tag="mod_qi")
            nc.vector.tensor_copy(qi[:], m[:])
            qf = ipool.tile([128, tiles], f32, tag="mod_qf")
            nc.vector.tensor_copy(qf[:], qi[:])
            q3 = ipool.tile([128, tiles], f32, tag="mod_q3")
            nc.vector.tensor_scalar(
                q3[:], qf[:], float(SZ), None, op0=mybir.AluOpType.mult
            )
            nc.vector.tensor_tensor(dst[:], src[:], q3[:], op=mybir.AluOpType.subtract)
            fix = ipool.tile([128, tiles], f32, tag="mod_fix")
            nc.vector.tensor_scalar(
                fix[:], dst[:], 0.0, float(SZ),
                op0=mybir.AluOpType.is_lt, op1=mybir.AluOpType.mult,
            )
            nc.vector.tensor_tensor(dst[:], dst[:], fix[:], op=mybir.AluOpType.add)
            nc.vector.tensor_scalar(
                fix[:], dst[:], float(SZ), float(-SZ),
                op0=mybir.AluOpType.is_ge, op1=mybir.AluOpType.mult,
            )
            nc.vector.tensor_tensor(dst[:], dst[:], fix[:], op=mybir.AluOpType.add)

        t_sb = ipool.tile([128, tiles], i32)
        nc.scalar.dma_start(t_sb[:], t_ap[:])
        tp_sb = ipool.tile([128, tiles], i32)
        nc.scalar.dma_start(tp_sb[:], tp_ap[:])
        base_sb = ipool.tile([128, 1], f32)
        nc.scalar.dma_start(base_sb[:], base_ap[:])

        tf = ipool.tile([128, tiles], f32)
        nc.vector.tensor_copy(tf[:], t_sb[:])
        pf = ipool.tile([128, tiles], f32)
        nc.vector.tensor_copy(pf[:], tp_sb[:])

        tm = ipool.tile([128, tiles], f32)
        mod_sz(tm, tf)
        pm = ipool.tile([128, tiles], f32)
        mod_sz(pm, pf)

        s_sb = ipool.tile([128, tiles], f32)
        # s = (t % SZ)*C_T + (prev % SZ)*C_P  (< 2^23, exact in fp32)
        nc.vector.tensor_scalar(tm[:], tm[:], float(C_T), None,
                                op0=mybir.AluOpType.mult)
        nc.vector.tensor_scalar(pm[:], pm[:], float(C_P), None,
                                op0=mybir.AluOpType.mult)
        nc.vector.tensor_tensor(s_sb[:], tm[:], pm[:], op=mybir.AluOpType.add)
        sf = ipool.tile([128, tiles], f32)
        mod_sz(sf, s_sb)
        # rebase into the local slice and clamp (pad tokens may fall
        # outside this core's slice; their rows are discarded by the host)
        nc.vector.tensor_tensor(sf[:], sf[:],
                                base_sb[:, 0:1].to_broadcast([128, tiles]),
                                op=mybir.AluOpType.subtract)
        nc.vector.tensor_scalar(sf[:], sf[:], 0.0, float(SLICE - 1),
                                op0=mybir.AluOpType.max, op1=mybir.AluOpType.min)
        idx_sb = ipool.tile([128, tiles], i32)
        nc.vector.tensor_copy(idx_sb[:], sf[:])

        # ---- fold: tab2[c*SLICE:(c+1)*SLICE] = tab[rows] @ w_proj.T ----
        for rc in range(RC_LOC if do_fold else 0):
            ps = ppool.tile([128, D], f32)
            for kc in range(KC):
                lhsT = tabT_sb[kc][:, rc * 128 : (rc + 1) * 128]
                nc.tensor.matmul(
                    ps[:, 0:512], lhsT, wT_sb[kc][:, 0:512],
                    start=(kc == 0), stop=(kc == KC - 1),
                )
                nc.tensor.matmul(
                    ps[:, 512:1024], lhsT, wT_sb[kc][:, 512:1024],
                    start=(kc == 0), stop=(kc == KC - 1),
                )
            fo = fpool.tile([128, D], f32)
            nc.vector.tensor_copy(fo[:], ps[:])
            nc.sync.dma_start(tab2_ap[rc * 128 : (rc + 1) * 128, :], fo[:])

        # ---- gather + write out ----
        # bmax[j]: highest 128-row fold chunk tile j touches (tokens are
        # sorted by index, so early tiles only need early fold chunks —
        # range-based dep tracking lets those gathers overlap later fold
        # chunks). None -> conservative full span.
        for j0 in range(0, tiles if do_gather else 0, out_chunk):
            k = min(out_chunk, tiles - j0)
            g = gpool.tile([128, k * D], f32)
            for jj in range(k):
                j = j0 + jj
                span = SLICE if bmax is None else 128 * (bmax[j] + 1)
                nc.gpsimd.indirect_dma_start(
                    out=g[:, jj * D : (jj + 1) * D],
                    out_offset=None,
                    in_=tab2_ap[0:span, :],
                    in_offset=bass.IndirectOffsetOnAxis(
                        ap=idx_sb[:, j : j + 1], axis=0
                    ),
                )
            nc.sync.dma_start(
                out_ap[j0 * 128 : (j0 + k) * 128, :].rearrange(
                    "(k p) d -> p (k d)", k=k
                ),
                g[:],
            )


def build(tiles, loop_iters=None, bmax=None, **body_kw):
    """Build the SPMD Bass program (same program for all 8 cores).

    tiles: number of 128-token gather tiles per core (capacity).
    loop_iters: if set, wrap the (idempotent) body in a For_i loop that
    executes it that many times — used only for timing amplification.
    """
    key = ("nc", tiles, loop_iters, bmax, tuple(sorted(body_kw.items())))
    if key in _CACHE:
        return _CACHE[key]
    nc = bacc.Bacc("TRN2", target_bir_lowering=False, debug=False)
    io = declare_io(nc, tiles)
    with tile.TileContext(nc) as tc:
        if loop_iters is None:
            emit_body(nc, tc, io, tiles, bmax=bmax, **body_kw)
        else:
            with tc.For_i(0, loop_iters, 1):
                emit_body(nc, tc, io, tiles, bmax=bmax, **body_kw)
    nc.compile()
    _CACHE[key] = nc
    return nc


def _hash_idx_host(t_flat, p_flat):
    a = (t_flat.astype(np.int64) % SZ) * C_T
    b = (p_flat.astype(np.int64) % SZ) * C_P
    return ((a + b) % SZ).astype(np.int64)


def route(t, tab=None, w_proj=None):
    """Host routing: order tokens by owning core; returns the order and
    per-core counts, plus the padded per-core capacity in 128-token tiles."""
    t = np.asarray(t)
    prev = np.pad(t[:, :-1], ((0, 0), (1, 0)))
    t_flat = np.ascontiguousarray(t, dtype=np.int32).reshape(-1)
    p_flat = np.ascontiguousarray(prev, dtype=np.int32).reshape(-1)
    idx = _hash_idx_host(t_flat, p_flat)
    owner = idx // SLICE
    # sort by full index == sort by (owner, local idx): per-core tokens
    # are then ordered by table row, so gather tile j only touches a
    # prefix of the fold chunks.
    order = np.argsort(idx, kind="stable")
    counts = np.bincount(owner, minlength=N_CORES)
    tiles = max(1, int(-(-counts.max() // 128)))
    return t_flat, p_flat, idx, order, counts, tiles


def make_in_maps(t, tab, w_proj):
    """Host-side marshalling: route tokens, shard table rows, transpose."""
    tab = np.ascontiguousarray(np.asarray(tab), dtype=np.float32)
    w_proj = np.ascontiguousarray(np.asarray(w_proj), dtype=np.float32)
    t_flat, p_flat, idx, order, counts, tiles = route(t)
    cap = tiles * 128

    tabT = np.ascontiguousarray(tab.T)                       # [D, SZ]
    wT = np.ascontiguousarray(w_proj.T).reshape(KC, 128, D)

    in_maps = []
    bmax_per_core = []
    off = 0
    for c in range(N_CORES):
        n = int(counts[c])
        toks = order[off : off + n]
        off += n
        t_sh = np.zeros(cap, np.int32)
        tp_sh = np.zeros(cap, np.int32)
        t_sh[:n] = t_flat[toks]
        tp_sh[:n] = p_flat[toks]
        loc = np.zeros(cap, np.int64)
        loc[:n] = idx[toks] - c * SLICE
        bm = tuple(
            int(loc[j * 128 : min((j + 1) * 128, n)].max() // 128)
            if j * 128 < n else 0
            for j in range(tiles)
        )
        bmax_per_core.append(bm)
        # device layout [128, tiles]: element [p, j] = slot j*128 + p
        t_sh = np.ascontiguousarray(t_sh.reshape(tiles, 128).T)
        tp_sh = np.ascontiguousarray(tp_sh.reshape(tiles, 128).T)
        base = np.full((128, 1), c * SLICE, np.float32)
        tabT_sl = np.ascontiguousarray(
            tabT[:, c * SLICE : (c + 1) * SLICE]
        ).reshape(KC, 128, SLICE)
        in_maps.append(
            {"t_sh": t_sh, "tp_sh": tp_sh, "base": base,
             "tabT": tabT_sl, "w_projT": wT}
        )
    # SPMD: one program for all cores — take the elementwise max over cores
    bmax = tuple(
        max(bmax_per_core[c][j] for c in range(N_CORES)) for j in range(tiles)
    )
    return in_maps, order, counts, tiles, bmax


def kernel(t, tab, w_proj):
    in_maps, order, counts, tiles, bmax = make_in_maps(t, tab, w_proj)
    nc = build(tiles, bmax=bmax)
    res = run_bass_kernel_spmd(nc, in_maps, list(range(N_CORES)))
    out = np.empty((NTOK, D), np.float32)
    off = 0
    for c in range(N_CORES):
        n = int(counts[c])
        out[order[off : off + n]] = res.results[c]["out_sh"][:n]
        off += n
    return out.reshape(B, T, D)


# revision 17
# speedup vs baseline: 1.1184x; 1.1184x over previous
"""Trainium2 Bass kernel for nn_BigramHash: out = tab[hash(t,prev)] @ w_proj.T.

Strategy (fold sharded by table rows, tokens routed to row owners):
  - The projection is folded into the table on-device:
        tab2 = tab @ w_proj.T
    sharded by table rows: core c computes rows [c*384, (c+1)*384).
  - The host routes each token to the core that owns its hashed row
    (the hash is recomputed on-device for the actual gather; the host
    copy is only the sharding function), padding each shard to a
    common capacity. Tokens are sorted by table row, which (a) gives
    the gather DRAM row-buffer locality and (b) lets early gather
    tiles depend only on the first fold chunks (range-based dep
    tracking), overlapping the gather with the fold.
  - Each core computes the bigram hash for its tokens on DVE (exact in
    fp32: all intermediates < 2^24), rebases it into its local slice,
    gathers rows with indirect DMA (128 rows x 4KB per instruction)
    and streams them to the output.
  - The host scatters per-core output rows back to token order.

Everything numeric from the reference (hash, fold matmul, gather) runs
on device; host work is sharding/layout marshalling and the routing
permutation. Output is bit-exact vs the fp32 reference.
"""

import numpy as np

import concourse.bass as bass
import concourse.tile as tile
from concourse import bacc, mybir
from concourse.bass_utils import run_bass_kernel_spmd

N_CORES = 8
B, T = 4, 8192
SZ, D = 3072, 1024
NTOK = B * T                      # 32768
SLICE = SZ // N_CORES             # 384 table rows per core
RC_LOC = SLICE // 128             # 3 fold row-chunks per core
KC = D // 128                     # 8 contraction chunks

C_T = 31337 % SZ                  # 617
C_P = 1000003 % SZ                # 1603

_CACHE = {}


def declare_io(nc, tiles):
    f32 = mybir.dt.float32
    i32 = mybir.dt.int32
    t_ap = nc.dram_tensor("t_sh", [128, tiles], i32, kind="ExternalInput").ap()
    tp_ap = nc.dram_tensor("tp_sh", [128, tiles], i32, kind="ExternalInput").ap()
    base_ap = nc.dram_tensor("base", [128, 1], f32, kind="ExternalInput").ap()
    tabT_ap = nc.dram_tensor(
        "tabT", [KC, 128, SLICE], f32, kind="ExternalInput"
    ).ap()
    wT_ap = nc.dram_tensor("w_projT", [KC, 128, D], f32, kind="ExternalInput").ap()
    out_ap = nc.dram_tensor("out_sh", [tiles * 128, D], f32, kind="ExternalOutput").ap()
    tab2_ap = nc.dram_tensor("tab2", [SLICE, D], f32).ap()
    return t_ap, tp_ap, base_ap, tabT_ap, wT_ap, out_ap, tab2_ap


def emit_body(nc, tc, io, tiles, bmax=None, do_fold=True, do_gather=True,
              gather_bufs=8, out_chunk=2, alt_rings=True):
    f32 = mybir.dt.float32
    i32 = mybir.dt.int32
    t_ap, tp_ap, base_ap, tabT_ap, wT_ap, out_ap, tab2_ap = io
    with (
        tc.tile_pool(name="weights", bufs=1) as wpool,
        tc.tile_pool(name="fold_out", bufs=3) as fpool,
        tc.tile_pool(name="psum", bufs=2, space="PSUM") as ppool,
        tc.tile_pool(name="idx", bufs=1) as ipool,
        tc.tile_pool(name="gather", bufs=gather_bufs) as gpool,
    ):
        # ---- load tabT slice (1.5 MB) and w_projT (4 MB) into SBUF ----
        tabT_sb = []
        wT_sb = []
        for kc in range(KC):
            tt = wpool.tile([128, SLICE], f32, tag=f"tabT{kc}")
            nc.scalar.dma_start(tt[:], tabT_ap[kc])
            tabT_sb.append(tt)
            wt = wpool.tile([128, D], f32, tag=f"wT{kc}")
            nc.scalar.dma_start(wt[:], wT_ap[kc])
            wT_sb.append(wt)

        # ---- hash indices on DVE ----
        # No integer mod in the TRN2 DVE ISA; compute x % SZ exactly in
        # fp32 (all intermediates < 2^24): q = int(x/SZ) may be off by
        # one in either direction, two masked corrections fix it.
        def mod_sz(dst, src):
            m = ipool.tile([128, tiles], f32, tag="mod_m")
            nc.vector.tensor_scalar(
                m[:], src[:], 1.0 / SZ, None, op0=mybir.AluOpType.mult
            )
            qi = ipool.tile([128, tiles], i32, tag="mod_qi")
            nc.vector.tensor_copy(qi[:], m[:])
            qf = ipool.tile([128, tiles], f32, tag="mod_qf")
            nc.vector.tensor_copy(qf[:], qi[:])
            q3 = ipool.tile([128, tiles], f32, tag="mod_q3")
            nc.vector.tensor_scalar(
                q3[:], qf[:], float(SZ), None, op0=mybir.AluOpType.mult
            )
            nc.vector.tensor_tensor(dst[:], src[:], q3[:], op=mybir.AluOpType.subtract)
            fix = ipool.tile([128, tiles], f32, tag="mod_fix")
            nc.vector.tensor_scalar(
                fix[:], dst[:], 0.0, float(SZ),
                op0=mybir.AluOpType.is_lt, op1=mybir.AluOpType.mult,
            )
            nc.vector.tensor_tensor(dst[:], dst[:], fix[:], op=mybir.AluOpType.add)
            nc.vector.tensor_scalar(
                fix[:], dst[:], float(SZ), float(-SZ),
                op0=mybir.AluOpType.is_ge, op1=mybir.AluOpType.mult,
            )
            nc.vector.tensor_tensor(dst[:], dst[:], fix[:], op=mybir.AluOpType.add)

        t_sb = ipool.tile([128, tiles], i32)
        nc.scalar.dma_start(t_sb[:], t_ap[:])
        tp_sb = ipool.tile([128, tiles], i32)
        nc.scalar.dma_start(tp_sb[:], tp_ap[:])
        base_sb = ipool.tile([128, 1], f32)
        nc.scalar.dma_start(base_sb[:], base_ap[:])

        tf = ipool.tile([128, tiles], f32)
        nc.vector.tensor_copy(tf[:], t_sb[:])
        pf = ipool.tile([128, tiles], f32)
        nc.vector.tensor_copy(pf[:], tp_sb[:])

        tm = ipool.tile([128, tiles], f32)
        mod_sz(tm, tf)
        pm = ipool.tile([128, tiles], f32)
        mod_sz(pm, pf)

        s_sb = ipool.tile([128, tiles], f32)
        # s = (t % SZ)*C_T + (prev % SZ)*C_P  (< 2^23, exact in fp32)
        nc.vector.tensor_scalar(tm[:], tm[:], float(C_T), None,
                                op0=mybir.AluOpType.mult)
        nc.vector.tensor_scalar(pm[:], pm[:], float(C_P), None,
                                op0=mybir.AluOpType.mult)
        nc.vector.tensor_tensor(s_sb[:], tm[:], pm[:], op=mybir.AluOpType.add)
        sf = ipool.tile([128, tiles], f32)
        mod_sz(sf, s_sb)
        # rebase into the local slice and clamp (pad tokens may fall
        # outside this core's slice; their rows are discarded by the host)
        nc.vector.tensor_tensor(sf[:], sf[:],
                                base_sb[:, 0:1].to_broadcast([128, tiles]),
                                op=mybir.AluOpType.subtract)
        nc.vector.tensor_scalar(sf[:], sf[:], 0.0, float(SLICE - 1),
                                op0=mybir.AluOpType.max, op1=mybir.AluOpType.min)
        idx_sb = ipool.tile([128, tiles], i32)
        nc.vector.tensor_copy(idx_sb[:], sf[:])

        # ---- fold: tab2[c*SLICE:(c+1)*SLICE] = tab[rows] @ w_proj.T ----
        for rc in range(RC_LOC if do_fold else 0):
            ps = ppool.tile([128, D], f32)
            for kc in range(KC):
                lhsT = tabT_sb[kc][:, rc * 128 : (rc + 1) * 128]
                nc.tensor.matmul(
                    ps[:, 0:512], lhsT, wT_sb[kc][:, 0:512],
                    start=(kc == 0), stop=(kc == KC - 1),
                )
                nc.tensor.matmul(
                    ps[:, 512:1024], lhsT, wT_sb[kc][:, 512:1024],
                    start=(kc == 0), stop=(kc == KC - 1),
                )
            fo = fpool.tile([128, D], f32)
            nc.vector.tensor_copy(fo[:], ps[:])
            nc.sync.dma_start(tab2_ap[rc * 128 : (rc + 1) * 128, :], fo[:])

        # ---- gather + write out ----
        # bmax[j]: highest 128-row fold chunk tile j touches (tokens are
        # sorted by index, so early tiles only need early fold chunks —
        # range-based dep tracking lets those gathers overlap later fold
        # chunks). None -> conservative full span.
        for j0 in range(0, tiles if do_gather else 0, out_chunk):
            k = min(out_chunk, tiles - j0)
            g = gpool.tile([128, k * D], f32)
            for jj in range(k):
                j = j0 + jj
                span = SLICE if bmax is None else 128 * (bmax[j] + 1)
                nc.gpsimd.indirect_dma_start(
                    out=g[:, jj * D : (jj + 1) * D],
                    out_offset=None,
                    in_=tab2_ap[0:span, :],
                    in_offset=bass.IndirectOffsetOnAxis(
                        ap=idx_sb[:, j : j + 1], axis=0
                    ),
                )
            out_eng = nc.scalar if (alt_rings and (j0 // out_chunk) % 2) else nc.sync
            out_eng.dma_start(
                out_ap[j0 * 128 : (j0 + k) * 128, :].rearrange(
                    "(k p) d -> p k d", k=k
                ),
                g[:].rearrange("p (k d) -> p k d", k=k),
            )


def build(tiles, loop_iters=None, bmax=None, **body_kw):
    """Build the SPMD Bass program (same program for all 8 cores).

    tiles: number of 128-token gather tiles per core (capacity).
    loop_iters: if set, wrap the (idempotent) body in a For_i loop that
    executes it that many times — used only for timing amplification.
    """
    key = ("nc", tiles, loop_iters, bmax, tuple(sorted(body_kw.items())))
    if key in _CACHE:
        return _CACHE[key]
    nc = bacc.Bacc("TRN2", target_bir_lowering=False, debug=False)
    io = declare_io(nc, tiles)
    with tile.TileContext(nc) as tc:
        if loop_iters is None:
            emit_body(nc, tc, io, tiles, bmax=bmax, **body_kw)
        else:
            with tc.For_i(0, loop_iters, 1):
                emit_body(nc, tc, io, tiles, bmax=bmax, **body_kw)
    nc.compile()
    _CACHE[key] = nc
    return nc


def _hash_idx_host(t_flat, p_flat):
    a = (t_flat.astype(np.int64) % SZ) * C_T
    b = (p_flat.astype(np.int64) % SZ) * C_P
    return ((a + b) % SZ).astype(np.int64)


def route(t, tab=None, w_proj=None):
    """Host routing: order tokens by owning core; returns the order and
    per-core counts, plus the padded per-core capacity in 128-token tiles."""
    t = np.asarray(t)
    prev = np.pad(t[:, :-1], ((0, 0), (1, 0)))
    t_flat = np.ascontiguousarray(t, dtype=np.int32).reshape(-1)
    p_flat = np.ascontiguousarray(prev, dtype=np.int32).reshape(-1)
    idx = _hash_idx_host(t_flat, p_flat)
    owner = idx // SLICE
    # sort by full index == sort by (owner, local idx): per-core tokens
    # are then ordered by table row, so gather tile j only touches a
    # prefix of the fold chunks.
    order = np.argsort(idx, kind="stable")
    counts = np.bincount(owner, minlength=N_CORES)
    tiles = max(1, int(-(-counts.max() // 128)))
    return t_flat, p_flat, idx, order, counts, tiles


def make_in_maps(t, tab, w_proj):
    """Host-side marshalling: route tokens, shard table rows, transpose."""
    tab = np.ascontiguousarray(np.asarray(tab), dtype=np.float32)
    w_proj = np.ascontiguousarray(np.asarray(w_proj), dtype=np.float32)
    t_flat, p_flat, idx, order, counts, tiles = route(t)
    cap = tiles * 128

    tabT = np.ascontiguousarray(tab.T)                       # [D, SZ]
    wT = np.ascontiguousarray(w_proj.T).reshape(KC, 128, D)

    in_maps = []
    bmax_per_core = []
    off = 0
    for c in range(N_CORES):
        n = int(counts[c])
        toks = order[off : off + n]
        off += n
        t_sh = np.zeros(cap, np.int32)
        tp_sh = np.zeros(cap, np.int32)
        t_sh[:n] = t_flat[toks]
        tp_sh[:n] = p_flat[toks]
        loc = np.zeros(cap, np.int64)
        loc[:n] = idx[toks] - c * SLICE
        bm = tuple(
            int(loc[j * 128 : min((j + 1) * 128, n)].max() // 128)
            if j * 128 < n else 0
            for j in range(tiles)
        )
        bmax_per_core.append(bm)
        # device layout [128, tiles]: element [p, j] = slot j*128 + p
        t_sh = np.ascontiguousarray(t_sh.reshape(tiles, 128).T)
        tp_sh = np.ascontiguousarray(tp_sh.reshape(tiles, 128).T)
        base = np.full((128, 1), c * SLICE, np.float32)
        tabT_sl = np.ascontiguousarray(
            tabT[:, c * SLICE : (c + 1) * SLICE]
        ).reshape(KC, 128, SLICE)
        in_maps.append(
            {"t_sh": t_sh, "tp_sh": tp_sh, "base": base,
             "tabT": tabT_sl, "w_projT": wT}
        )
    # SPMD: one program for all cores — take the elementwise max over cores
    bmax = tuple(
        max(bmax_per_core[c][j] for c in range(N_CORES)) for j in range(tiles)
    )
    return in_maps, order, counts, tiles, bmax


def kernel(t, tab, w_proj):
    in_maps, order, counts, tiles, bmax = make_in_maps(t, tab, w_proj)
    nc = build(tiles, bmax=bmax)
    res = run_bass_kernel_spmd(nc, in_maps, list(range(N_CORES)))
    out = np.empty((NTOK, D), np.float32)
    off = 0
    for c in range(N_CORES):
        n = int(counts[c])
        out[order[off : off + n]] = res.results[c]["out_sh"][:n]
        off += n
    return out.reshape(B, T, D)
